# revision 1
# baseline (speedup 1.0000x reference)
"""Trainium2 Bass kernel for MemoryEfficientCrossAttention (v2).

Problem (hardcoded): B=2, Q=2048, K=4096, HIDDEN=1024, HEADS=16, HEAD_DIM=64.
  out = softmax((x_q W_q)(x_k W_k)^T / sqrt(64)) (x_v W_v) W_o

Sharding over 8 NeuronCores: core = b*4 + t
  b in {0,1}: batch;  t in {0..3}: head-quad (4 heads = 256 cols of W_q/k/v).
No duplicated FLOPs: each core projects q/k/v for its quad only, runs
attention for (full Q x its 4 heads), computes the partial out projection
ctx_t @ W_o[256t:256t+256, :], and a 4-way ReduceScatter sums the partials
(rank r keeps q-rows r*128..r*128+128 of each 512-row block).

Host passes activations PRE-TRANSPOSED in bf16 (xT layout [HID, rows]) so the
kernel needs no on-chip transposes or fp32->bf16 conversion; W_q carries the
softmax scale (and an extra 1/8 * c1 so scores arrive as z = c1*s/8, the
argument of the DVE poly-exp).  All matmuls run bf16 x bf16 -> fp32 PSUM
(full PE rate).

Scores land transposed (S^T[k,q]) per head-pair in [128,2,512] PSUM tiles;
exp runs on the Act engine (scale=8/c1) and, for a tunable fraction of
tiles, on the DVE via two fused custom ops (quartic poly -> fp16, then ^8
-> bf16).  PV accumulates ctx^T [q, head, d|r] in single-bank PSUM scratch
per 8-kb group (ones column in V gives the softmax denominator r), flushed
to an SBUF fp32 accumulator by the DVE; 1/r is then a per-partition scalar
multiply (no partition broadcast).  ctx^T is xbar-transposed (bf16) for the
output projection, whose per-q-block partials ReduceScatter across the
4 cores of the batch.
"""

import os
import sys
import time

import numpy as np

sys.path.insert(0, "/opt/trn_rl_repo")

from contextlib import ExitStack  # noqa: E402

import concourse.bass as bass  # noqa: E402
import concourse.mybir as mybir  # noqa: E402
import concourse.tile as tile  # noqa: E402
from concourse import bacc  # noqa: E402

F32 = mybir.dt.float32
BF16 = mybir.dt.bfloat16
FP16 = mybir.dt.float16

HID = 1024
HEADS = 16
HD = 64
B = 2
Q = 2048
KL = 4096
NCORE = 8
TC = 256            # head-quad cols per core (4 heads)
NCH = HID // 128    # 8 hidden chunks
NKB = KL // 128     # 32 k-blocks
NQB = Q // 512      # 4 q-blocks of 512
GRP = 8             # k-blocks per PV scratch group
SCALE = HD ** -0.5

# exp(s) = q(z)^8, z = c1*s/8:  q = 1 + z + A z^2 + B z^3 + C z^4
EXP_C1 = 0.9993923
_c1, _c2, _c3, _c4 = 0.9993923, 0.5014591, 0.17676774, 0.04129203
EXP_A = _c2 / _c1 ** 2
EXP_B = _c3 / _c1 ** 3
EXP_CC = _c4 / _c1 ** 4

# Of every 8 (kb, pair) exp tiles, this many go to the DVE fast-exp.
DVE_EXP_NUM = int(os.environ.get("KDVE_EXP_NUM", "0"))
# Cross-core reduction of out-projection partials happens on the HOST
# (collective_compute costs 150-450us each on this runtime).
_DVE_OPS = None


def _get_dve_exp_ops():
    """Register the fused DVE ops: quartic poly and ^8."""
    global _DVE_OPS
    if _DVE_OPS is not None:
        return _DVE_OPS
    from concourse import dve_ops as dops
    from concourse.dve_spec import (Spec, Src0, C0, C1, C2, One, sq, lower)
    from concourse.dve_uop import DveOpSpec
    from concourse.dve_table_gen import dve_ver_for

    def register(name, spec):
        for op in dops.OPS:
            if op.name == name:
                return op
        opcode = dops._CUSTOM_DVE_ROW_BASE + len(dops.OPS)
        shas = {}
        for ver in ("v3", "v4"):
            s = DveOpSpec(name=name, opcode=opcode,
                          uops=lower(spec, ver=ver), rd1_en=False)
            shas[ver] = s.sha(ver)
        op = dops.DveOp(name, spec, subdim=False, uops_sha=shas)
        dops.OPS.append(op)
        dops.CUSTOM_DVE_SPECS[name] = spec
        dops._SUB_OPCODE_FOR_NAME[name] = opcode
        return op

    # q(z) = (1 + z) + z^2*((A + B z) + C z^2)   [8 ALU ops]
    z2 = sq(Src0)
    inner = (C0 + C1 * Src0) + C2 * z2
    qpoly = (One + Src0) + z2 * inner

    def ref_q(in0, in1, s0, s1, imm2):
        z = np.asarray(in0, np.float32).astype(np.float64)
        return 1.0 + z + z * z * (s0 + s1 * z + imm2 * z * z)

    def ref_p8(in0, in1, s0, s1, imm2):
        x = np.asarray(in0, np.float32).astype(np.float64)
        return x ** 8

    q_op = register("EXPQ4_ANT", Spec(body=qpoly, reference=ref_q))
    p8_op = register("POW8_ANT",
                     Spec(body=sq(sq(sq(Src0))), reference=ref_p8))
    _DVE_OPS = (q_op, p8_op)
    return _DVE_OPS


_CACHED_NC = None


def _build():
    use_dve_exp = DVE_EXP_NUM > 0
    if use_dve_exp:
        q_op, p8_op = _get_dve_exp_ops()

    nc = bacc.Bacc("TRN2", target_bir_lowering=False, debug=False,
                   num_devices=NCORE)

    xqT = nc.dram_tensor("xqT", [HID, Q], BF16, kind="ExternalInput")
    xkT = nc.dram_tensor("xkT", [HID, KL], BF16, kind="ExternalInput")
    xvT = nc.dram_tensor("xvT", [HID, KL], BF16, kind="ExternalInput")
    wq = nc.dram_tensor("wq", [HID, TC], BF16, kind="ExternalInput")
    wk = nc.dram_tensor("wk", [HID, TC], BF16, kind="ExternalInput")
    wv = nc.dram_tensor("wv", [HID, TC], BF16, kind="ExternalInput")
    wo = nc.dram_tensor("wo", [TC, HID], BF16, kind="ExternalInput")
    o_part = nc.dram_tensor("o_part", [Q, HID], BF16, kind="ExternalOutput")

    with tile.TileContext(nc, pool_alloc_mode="queue") as tc:
        with tc.tile_pool(name="dram", bufs=1, space="DRAM") as dram:
            _pools = ExitStack()
            pp = _pools.enter_context(tc.tile_pool(name="persist", bufs=1))
            wq_sb = pp.tile([128, NCH, TC], BF16)
            wk_sb = pp.tile([128, NCH, TC], BF16)
            wv_sb = pp.tile([128, NCH, TC], BF16)
            wo_sb = pp.tile([128, 2, HID], BF16)
            qT = pp.tile([128, 2, Q], BF16)
            kT = pp.tile([128, 2, KL], BF16)
            v_aug = pp.tile([128, NKB, 4, HD + 1], BF16)
            # ctxacc[q, qq16, h, e]: fp32 ctx^T accumulator (e==64 is r)
            ctxacc = pp.tile([128, 16, 4, HD + 1], F32)
            ctxT = pp.tile([128, 16, TC], BF16)
            # ctx2[p, qb, qq, j, r]: cd=j*128+p on partitions, q free
            ctx2 = pp.tile([128, NQB, 4, 2, 128], BF16)

            xs = _pools.enter_context(tc.tile_pool(name="xstage", bufs=2))
            apool = _pools.enter_context(
                tc.tile_pool(name="apool", bufs=2 * GRP + 10))
            ostg = _pools.enter_context(tc.tile_pool(name="ostg", bufs=2))
            misc = _pools.enter_context(tc.tile_pool(name="misc", bufs=2))
            etmp = _pools.enter_context(tc.tile_pool(name="etmp", bufs=3))

            nc.vector.memset(v_aug[:, :, :, HD], 1.0)

            # ---------------- weight loads (one DMA each) ----------------
            nc.sync.dma_start(wq_sb[:], wq[:, :].rearrange(
                "(c p) n -> p c n", p=128))
            nc.sync.dma_start(wk_sb[:], wk[:, :].rearrange(
                "(c p) n -> p c n", p=128))
            nc.sync.dma_start(wv_sb[:], wv[:, :].rearrange(
                "(c p) n -> p c n", p=128))
            nc.sync.dma_start(wo_sb[:], wo[:, :].rearrange(
                "(j p) n -> p j n", p=128))

            def load_xslab(src, s):
                """[128, NCH, 512] bf16 slab of pre-transposed activations."""
                xsl = xs.tile([128, NCH, 512], BF16, tag="xsl", name="xsl")
                nc.sync.dma_start(
                    xsl[:], src[:, s * 512:(s + 1) * 512]
                    .rearrange("(c p) r -> p c r", p=128))
                return xsl

            with (
                tc.tile_pool(name="pst", bufs=2, space="PSUM") as pst,
                tc.tile_pool(name="pscr", bufs=2, space="PSUM") as pscr,
            ):
                def proj_strips(xsl, w_sb, dst, dsl, pj):
                    for i in range(2):
                        ps = pj.tile([128, 512], F32, tag="pjt", name="pjt")
                        for hc in range(NCH):
                            nc.tensor.matmul(
                                ps[:], w_sb[:, hc, i * 128:(i + 1) * 128],
                                xsl[:, hc, :],
                                start=(hc == 0), stop=(hc == NCH - 1))
                        nc.vector.tensor_copy(dst[:, i, dsl], ps[:])

                def emit_v_slab(s, pj):
                    xsl = load_xslab(xvT, s)
                    for r in range(4):   # 128-row subtiles
                        ps = pj.tile([128, TC], F32, tag="pjt", name="pjv")
                        for hc in range(NCH):
                            nc.tensor.matmul(
                                ps[:], xsl[:, hc, r * 128:(r + 1) * 128],
                                wv_sb[:, hc, :],
                                start=(hc == 0), stop=(hc == NCH - 1))
                        kb = s * 4 + r
                        nc.vector.tensor_copy(
                            v_aug[:, kb, :, 0:HD],
                            ps[:].rearrange("p (h d) -> p h d", h=4))

                def emit_k_slab(s, pj):
                    xsl = load_xslab(xkT, s)
                    proj_strips(xsl, wk_sb, kT,
                                slice(s * 512, (s + 1) * 512), pj)

                def emit_exp(st, kb, pair):
                    """exp of score tile -> bf16 A tile [128, 2, 512]."""
                    a = apool.tile([128, 2, 512], BF16, tag="A",
                                   name=f"A_{kb}_{pair}")
                    if use_dve_exp and (kb * 2 + pair) % 8 < DVE_EXP_NUM:
                        t = etmp.tile([128, 2, 512], FP16, tag="et",
                                      name="et")
                        nc.vector._custom_dve(
                            q_op, out=t[:], in0=st[:],
                            s0=EXP_A, s1=EXP_B, imm2=EXP_CC)
                        nc.vector._custom_dve(p8_op, out=a[:], in0=t[:])
                    else:
                        nc.scalar.activation(
                            a[:], st[:], mybir.ActivationFunctionType.Exp,
                            scale=8.0 / EXP_C1)
                    return a

                pending_pv = []

                def make_pv_closure(qb, g, As):
                    def run_qq(qq):
                        scr = pscr.tile([128, 4, HD + 1], F32, tag="scr",
                                        name=f"scr{qb}_{g}_{qq}")
                        qqs = slice(qq * 128, (qq + 1) * 128)
                        for i in range(GRP):
                            for pair in range(2):
                                for j in range(2):
                                    h = 2 * pair + j
                                    nc.tensor.matmul(
                                        scr[:, h, :],
                                        As[i][pair][:, j, qqs],
                                        v_aug[:, g * GRP + i, h, :],
                                        start=(i == 0 and h == 0),
                                        stop=(i == GRP - 1 and h == 3))
                        qqg = qb * 4 + qq
                        if g == 0:
                            nc.vector.tensor_copy(
                                ctxacc[:, qqg, :, :], scr[:])
                        else:
                            nc.vector.tensor_add(
                                ctxacc[:, qqg, :, :],
                                ctxacc[:, qqg, :, :], scr[:])
                    return [lambda qq=qq: run_qq(qq) for qq in range(4)]

                def finish_qb(qb):
                    """1/r scale, bf16 convert, transpose, out projection."""
                    for qq in range(4):
                        qqg = qb * 4 + qq
                        rinv = misc.tile([128, 4, 1], F32, tag="rinv",
                                         name="rinv")
                        nc.vector.reciprocal(
                            rinv[:], ctxacc[:, qqg, :, HD:HD + 1])
                        for h in range(4):
                            nc.vector.tensor_scalar_mul(
                                ctxT[:, qqg, h * HD:(h + 1) * HD],
                                ctxacc[:, qqg, h, 0:HD],
                                rinv[:, h, :])
                    nc.sync.dma_start_transpose(
                        ctx2[:, qb], ctxT[:, qb * 4:(qb + 1) * 4, :])
                    emit_outproj(qb)

                def attention_sweep(qb, interleave):
                    qsl = slice(qb * 512, (qb + 1) * 512)
                    for g in range(NKB // GRP):
                        As = []
                        for kb in range(g * GRP, (g + 1) * GRP):
                            for fn in interleave.pop(kb, []):
                                fn()
                            if pending_pv:
                                pending_pv.pop(0)()
                            a_pair = []
                            for pair in range(2):
                                st = pst.tile([128, 2, 512], F32, tag="st",
                                              name=f"st{qb}_{kb}_{pair}")
                                ksl = slice(kb * 128, (kb + 1) * 128)
                                for j in range(2):
                                    psl = slice(j * 64, (j + 1) * 64)
                                    nc.tensor.matmul(
                                        st[:, j, :], kT[psl, pair, ksl],
                                        qT[psl, pair, qsl],
                                        start=True, stop=True)
                                a_pair.append(emit_exp(st, kb, pair))
                            As.append(a_pair)
                        pending_pv.extend(make_pv_closure(qb, g, As))

                def emit_outproj(qb):
                    for qq in range(4):
                        ob = ostg.tile([128, HID], BF16, tag="ob", name="ob")
                        for half in range(2):
                            po = pout[0].tile([128, 512], F32, tag="po",
                                              name="po")
                            osl = slice(half * 512, (half + 1) * 512)
                            for j in range(2):
                                nc.tensor.matmul(
                                    po[:], ctx2[:, qb, qq, j, :],
                                    wo_sb[:, j, osl],
                                    start=(j == 0), stop=(j == 1))
                            nc.vector.tensor_copy(ob[:, osl], po[:])
                        r0 = qb * 512 + qq * 128
                        nc.sync.dma_start(o_part[r0:r0 + 128, :], ob[:])

                pout = [None]
                with tc.tile_pool(name="pj", bufs=2, space="PSUM") as pj:
                    # q projection (4 slabs)
                    for s in range(4):
                        xsl = load_xslab(xqT, s)
                        proj_strips(xsl, wq_sb, qT,
                                    slice(s * 512, (s + 1) * 512), pj)
                    # kv slab 0 up front; rest interleaved into qb0's sweep
                    emit_k_slab(0, pj)
                    emit_v_slab(0, pj)
                    interleave = {}
                    for s in range(1, 8):
                        interleave.setdefault(4 * (s - 1), []).extend(
                            [lambda s=s: emit_k_slab(s, pj),
                             lambda s=s: emit_v_slab(s, pj)])
                    attention_sweep(0, interleave)

                with tc.tile_pool(name="pout", bufs=2, space="PSUM") as po_:
                    pout[0] = po_
                    for qb in range(1, NQB):
                        attention_sweep(
                            qb, {5: [lambda qb=qb: finish_qb(qb - 1)]})
                    while pending_pv:
                        pending_pv.pop(0)()
                    finish_qb(NQB - 1)

            _pools.close()

    nc.compile()
    return nc


def _get_nc():
    global _CACHED_NC
    if _CACHED_NC is None:
        _CACHED_NC = _build()
    return _CACHED_NC


def make_in_maps(query, key, value, w_q, w_k, w_v, w_o):
    import ml_dtypes
    bf = ml_dtypes.bfloat16
    qs = SCALE / 8.0 * EXP_C1   # softmax scale, /8 and *c1 for the poly-exp
    xq = [np.ascontiguousarray(query[b].T.astype(bf)) for b in range(B)]
    xk = [np.ascontiguousarray(key[b].T.astype(bf)) for b in range(B)]
    xv = [np.ascontiguousarray(value[b].T.astype(bf)) for b in range(B)]
    ins = []
    for core in range(NCORE):
        b, t = core // 4, core % 4
        csl = slice(t * TC, (t + 1) * TC)
        ins.append({
            "xqT": xq[b],
            "xkT": xk[b],
            "xvT": xv[b],
            "wq": np.ascontiguousarray((w_q[:, csl] * qs).astype(bf)),
            "wk": np.ascontiguousarray(w_k[:, csl].astype(bf)),
            "wv": np.ascontiguousarray(w_v[:, csl].astype(bf)),
            "wo": np.ascontiguousarray(w_o[csl, :].astype(bf)),
        })
    return ins


def assemble(results):
    out = np.empty((B, Q, HID), np.float32)
    for b in range(B):
        acc = results[b * 4]["o_part"].astype(np.float32)
        for t in range(1, 4):
            acc += results[b * 4 + t]["o_part"].astype(np.float32)
        out[b] = acc
    return out


_EXEC = None


def _get_exec():
    """Build the 8-core shard_map executable once; reuse across calls."""
    global _EXEC
    if _EXEC is not None:
        return _EXEC
    import jax
    from jax.sharding import Mesh, PartitionSpec
    from jax.experimental.shard_map import shard_map
    from concourse.bass2jax import (_bass_exec_p, install_neuronx_cc_hook,
                                    partition_id_tensor)

    install_neuronx_cc_hook()
    nc = _get_nc()
    in_names, out_names, out_avals, zero_outs = [], [], [], []
    for alloc in nc.m.functions[0].allocations:
        if not isinstance(alloc, mybir.MemoryLocationSet):
            continue
        name = alloc.memorylocations[0].name
        if alloc.kind == "ExternalInput":
            if name != "partition_id":
                in_names.append(name)
        elif alloc.kind == "ExternalOutput":
            out_names.append(name)
            shape = tuple(alloc.tensor_shape)
            dtype = mybir.dt.np(alloc.dtype)
            out_avals.append(jax.core.ShapedArray(shape, dtype))
            zero_outs.append(np.zeros(shape, dtype))
    partition_name = (nc.partition_id_tensor.name
                      if nc.partition_id_tensor else None)
    all_in = list(in_names) + list(out_names)
    if partition_name:
        all_in.append(partition_name)

    def _body(*args):
        operands = list(args)
        if partition_name is not None:
            operands.append(partition_id_tensor())
        return tuple(_bass_exec_p.bind(
            *operands, out_avals=tuple(out_avals), in_names=tuple(all_in),
            out_names=tuple(out_names), lowering_input_output_aliases=(),
            sim_require_finite=True, sim_require_nnan=True, nc=nc))

    devices = jax.devices()[:NCORE]
    mesh = Mesh(np.asarray(devices), ("core",))
    n_all = len(in_names) + len(out_names)
    fn = jax.jit(shard_map(_body, mesh=mesh,
                           in_specs=(PartitionSpec("core"),) * n_all,
                           out_specs=(PartitionSpec("core"),) * len(out_names),
                           check_rep=False), keep_unused=True)
    concat_zeros = [np.zeros((NCORE * z.shape[0], *z.shape[1:]), z.dtype)
                    for z in zero_outs]
    _EXEC = (fn, in_names, out_names, out_avals, concat_zeros)
    return _EXEC


def kernel(query, key, value, w_q, w_k, w_v, w_o):
    query = np.asarray(query, dtype=np.float32)
    key = np.asarray(key, dtype=np.float32)
    value = np.asarray(value, dtype=np.float32)
    ins = make_in_maps(query, key, value, np.asarray(w_q, np.float32),
                       np.asarray(w_k, np.float32),
                       np.asarray(w_v, np.float32),
                       np.asarray(w_o, np.float32))
    fn, in_names, out_names, out_avals, concat_zeros = _get_exec()
    concat_in = [np.concatenate([np.asarray(ins[c][nm]) for c in range(NCORE)])
                 for nm in in_names]
    out_arrs = fn(*concat_in, *concat_zeros)
    results = [
        {nm: np.asarray(out_arrs[i]).reshape(NCORE, *out_avals[i].shape)[c]
         for i, nm in enumerate(out_names)}
        for c in range(NCORE)]
    return assemble(results)


if __name__ == "__main__":
    np.random.seed(0)
    q = np.random.randn(B, Q, HID).astype(np.float32)
    k = np.random.randn(B, KL, HID).astype(np.float32)
    v = np.random.randn(B, KL, HID).astype(np.float32)
    s = 1.0 / np.sqrt(HID)
    wq_ = (np.random.randn(HID, HID) * s).astype(np.float32)
    wk_ = (np.random.randn(HID, HID) * s).astype(np.float32)
    wv_ = (np.random.randn(HID, HID) * s).astype(np.float32)
    wo_ = (np.random.randn(HID, HID) * s).astype(np.float32)
    t0 = time.time()
    out = kernel(q, k, v, wq_, wk_, wv_, wo_)
    print("kernel done", time.time() - t0, out.shape)



# revision 9
# speedup vs baseline: 4.7631x; 4.7631x over previous
"""Trainium2 Bass kernel for MemoryEfficientCrossAttention (v3).

Problem (hardcoded): B=2, Q=2048, K=4096, HIDDEN=1024, HEADS=16, HEAD_DIM=64.
  out = softmax((x_q W_q)(x_k W_k)^T / sqrt(64)) (x_v W_v) W_o

Sharding over 8 NeuronCores: core = b*4 + t
  b in {0,1}: batch;  t in {0..3}: head-quad (4 heads = 256 cols of W_q/k/v).
No duplicated FLOPs: each core projects q/k/v for its quad only, runs
attention for (full Q x its 4 heads), computes the partial out projection
ctx_t @ W_o[256t:256t+256, :]; the host sums the 4 partials per batch.

v3 structure (vs v2):
- Score PSUM tiles pack all 4 heads for a 256-query half-block
  [128 k, 4 h, 256 q] (2 banks); per bank the second head's matmul rides the
  first head's start=True bank-clear (start/stop per bank pair).  One Act
  exp instruction per tile (1024 free elems) -> A tile in SBUF bf16.
- 3-deep score-tile rotation (6 PSUM banks) gives the PE a 1.5-kblock
  runway over the Act engine, hiding cross-engine semaphore latency.
- The Act engine runs ONLY exp (single activation table load); every
  PSUM->SBUF copy, flush, and scale sits on the DVE.
- ctx^T for the out-projection comes from PE is_transpose matmuls (fp32,
  via an identity stationary) instead of the SP-heavy DMA transpose.
- K/V projections are spread as small closures across qb0's k-loop
  (just-in-time, 2-kblock DMA lead); the out-projection of qb is spread
  across qb+1's k-loop in per-qq pieces.
- PSUM: 3x2 banks score rotation + a shared 2-slot 1-bank "aux" ring
  (projection strips, PV scratch, transposes, out-proj accumulators).
"""

import sys
import time

import numpy as np

sys.path.insert(0, "/opt/trn_rl_repo")

from contextlib import ExitStack  # noqa: E402

import concourse.bass as bass  # noqa: E402
import concourse.mybir as mybir  # noqa: E402
import concourse.tile as tile  # noqa: E402
from concourse import bacc  # noqa: E402

F32 = mybir.dt.float32
BF16 = mybir.dt.bfloat16

HID = 1024
HEADS = 16
HD = 64
B = 2
Q = 2048
KL = 4096
NCORE = 8
TC = 256            # head-quad cols per core (4 heads)
NCH = HID // 128    # 8 hidden chunks
NKB = KL // 128     # 32 k-blocks
NQB = Q // 512      # 4 q-blocks of 512
HALF = 256          # queries per score tile
GRP = 8             # k-blocks per PV scratch group
SCALE = HD ** -0.5

EXP = mybir.ActivationFunctionType.Exp

_CACHED_NC = None


def _build():
    nc = bacc.Bacc("TRN2", target_bir_lowering=False, debug=False,
                   num_devices=NCORE)

    # All tensors arrive pre-arranged on the host so every DMA is contiguous
    # per partition (slab-major activations, partition-major weights).
    xqT = nc.dram_tensor("xqT", [4, 128, NCH, 512], BF16,
                         kind="ExternalInput")
    xkT = nc.dram_tensor("xkT", [8, 128, NCH, 512], BF16,
                         kind="ExternalInput")
    xvT = nc.dram_tensor("xvT", [8, 128, NCH, 512], BF16,
                         kind="ExternalInput")
    wq = nc.dram_tensor("wq", [128, NCH, TC], BF16, kind="ExternalInput")
    wk = nc.dram_tensor("wk", [128, NCH, TC], BF16, kind="ExternalInput")
    wv = nc.dram_tensor("wv", [128, NCH, TC], BF16, kind="ExternalInput")
    wo = nc.dram_tensor("wo", [128, 2, HID], BF16, kind="ExternalInput")
    eye = nc.dram_tensor("eye", [128, 128], F32, kind="ExternalInput")
    # o_part[qb, p, qq, :] = out_partial[qb*512 + qq*128 + p, :]
    o_part = nc.dram_tensor("o_part", [NQB, 128, 4, HID], BF16,
                            kind="ExternalOutput")

    with tile.TileContext(nc, pool_alloc_mode="queue") as tc:
        _pools = ExitStack()
        pp = _pools.enter_context(tc.tile_pool(name="persist", bufs=1))
        wq_sb = pp.tile([128, NCH, TC], BF16)
        wk_sb = pp.tile([128, NCH, TC], BF16)
        wv_sb = pp.tile([128, NCH, TC], BF16)
        wo_sb = pp.tile([128, 2, HID], BF16)
        eye_sb = pp.tile([128, 128], F32)
        qT = pp.tile([128, 2, Q], BF16)
        kT = pp.tile([128, 2, KL], BF16)
        v_aug = pp.tile([128, NKB, 4, HD + 1], BF16)
        # ctxacc[q, qq16, h, e]: fp32 ctx^T accumulator (e==64 is r)
        ctxacc = pp.tile([128, 16, 4, HD + 1], F32)

        xs = _pools.enter_context(tc.tile_pool(name="xstage", bufs=3))
        apool = _pools.enter_context(tc.tile_pool(name="apool", bufs=36))
        mp = _pools.enter_context(tc.tile_pool(name="misc", bufs=2))
        ps = _pools.enter_context(
            tc.tile_pool(name="ps", bufs=2, space="PSUM"))

        nc.vector.memset(v_aug[:, :, :, HD], 1.0)

        # ---------------- weight loads (one DMA each) ----------------
        nc.sync.dma_start(eye_sb[:], eye[:, :])
        nc.sync.dma_start(wq_sb[:], wq[:, :, :])
        nc.sync.dma_start(wk_sb[:], wk[:, :, :])
        nc.sync.dma_start(wv_sb[:], wv[:, :, :])
        nc.sync.dma_start(wo_sb[:], wo[:, :, :])

        def load_xslab(src, s):
            """[128, NCH, 512] bf16 slab of pre-transposed activations."""
            xsl = xs.tile([128, NCH, 512], BF16, tag="xsl", name="xsl")
            nc.sync.dma_start(xsl[:], src[s])
            return xsl

        def proj_strip(xsl, w_sb, dst, dsl, i):
            """One 512-col strip of a q/k projection into dst[:, i, dsl]."""
            pj = ps.tile([128, 512], F32, tag="aux", name="pjt")
            for hc in range(NCH):
                nc.tensor.matmul(
                    pj[:], w_sb[:, hc, i * 128:(i + 1) * 128],
                    xsl[:, hc, :],
                    start=(hc == 0), stop=(hc == NCH - 1))
            nc.vector.tensor_copy(dst[:, i, dsl], pj[:])

        def v_group(xsl, s, r):
            """One 128-row v-projection group -> v_aug[:, 4s+r]."""
            pj = ps.tile([128, TC], F32, tag="aux", name="pjv",
                         padded_shape=[128, 512])
            for hc in range(NCH):
                nc.tensor.matmul(
                    pj[:], xsl[:, hc, r * 128:(r + 1) * 128],
                    wv_sb[:, hc, :],
                    start=(hc == 0), stop=(hc == NCH - 1))
            nc.vector.tensor_copy(
                v_aug[:, s * 4 + r, :, 0:HD],
                pj[:].rearrange("p (h d) -> p h d", h=4))

        pending_pv = []
        a_tiles = {}

        # Score-tile slot order groups same-PE-row heads per PSUM bank:
        # bank0 = both j=0 heads, bank1 = both j=1 heads.  A start=False
        # matmul may not change the PE tile row mid-group on real HW.
        SLOT2HEAD = [0, 2, 1, 3]
        HEAD2SLOT = [0, 2, 1, 3]

        def emit_scores(qb, kb, half):
            st = ps.tile([128, 4, HALF], F32, tag="st", bufs=3,
                         name=f"st{qb}_{kb}_{half}")
            qsl = slice(qb * 512 + half * HALF, qb * 512 + (half + 1) * HALF)
            ksl = slice(kb * 128, (kb + 1) * 128)
            for slot in range(4):
                h = SLOT2HEAD[slot]
                pair, j = h // 2, h % 2
                psl = slice(j * 64, (j + 1) * 64)
                nc.tensor.matmul(
                    st[:, slot, :], kT[psl, pair, ksl], qT[psl, pair, qsl],
                    start=(slot % 2 == 0), stop=(slot % 2 == 1))
            a = apool.tile([128, 4, HALF], BF16, tag="A",
                           name=f"A{qb}_{kb}_{half}")
            nc.scalar.activation(a[:], st[:], EXP)
            a_tiles[(kb, half)] = a

        def make_pv_closures(qb, g):
            def run_qq(qq):
                scr = ps.tile([128, 4, HD + 1], F32, tag="aux",
                              name=f"scr{qb}_{g}_{qq}",
                              padded_shape=[128, 4, 128])
                half, qoff = qq // 2, (qq % 2) * 128
                for i in range(GRP):
                    kb = g * GRP + i
                    a = a_tiles[(kb, half)]
                    for h in range(4):
                        nc.tensor.matmul(
                            scr[:, h, :],
                            a[:, HEAD2SLOT[h], qoff:qoff + 128],
                            v_aug[:, kb, h, :],
                            start=(i == 0 and h == 0),
                            stop=(i == GRP - 1 and h == 3))
                qqg = qb * 4 + qq
                if g == 0:
                    nc.vector.tensor_copy(ctxacc[:, qqg, :, :], scr[:])
                else:
                    nc.vector.tensor_add(
                        ctxacc[:, qqg, :, :], ctxacc[:, qqg, :, :], scr[:])
            return [lambda qq=qq: run_qq(qq) for qq in range(4)]

        def finish_pieces(qb):
            """Per-qb epilogue as closures: 1/r scale (DVE), transpose (PE),
            out projection, staging copy, one output DMA."""
            ctxTs = [None] * 4
            ctx2s = [None] * 4
            cell = {}

            def piece_a(qq):
                qqg = qb * 4 + qq
                rinv = mp.tile([128, 4, 1], F32, tag="rinv", name="rinv")
                nc.vector.reciprocal(rinv[:], ctxacc[:, qqg, :, HD:HD + 1])
                ctxT = mp.tile([128, 4, HD], F32, tag="ctxT", name="ctxT",
                               bufs=3)
                for h in range(4):
                    nc.vector.tensor_scalar_mul(
                        ctxT[:, h, :], ctxacc[:, qqg, h, 0:HD],
                        rinv[:, h, :])
                ctxTs[qq] = ctxT

            def piece_b(qq):
                c2 = mp.tile([128, 2, 128], BF16, tag="ctx2", name="ctx2",
                             bufs=6)
                for j in range(2):
                    tp = ps.tile([128, 128], F32, tag="aux", name="tp",
                                 padded_shape=[128, 512])
                    nc.tensor.transpose(
                        tp[:], ctxTs[qq][:, 2 * j:2 * j + 2, :], eye_sb[:])
                    nc.vector.tensor_copy(c2[:, j, :], tp[:])
                ctx2s[qq] = c2

            def piece_c(qq, halfo):
                if "ob" not in cell:
                    cell["ob"] = mp.tile([128, 4, HID], BF16, tag="ob",
                                         name="ob")
                po = ps.tile([128, 512], F32, tag="aux", name="po")
                osl = slice(halfo * 512, (halfo + 1) * 512)
                for j in range(2):
                    nc.tensor.matmul(
                        po[:], ctx2s[qq][:, j, :], wo_sb[:, j, osl],
                        start=(j == 0), stop=(j == 1))
                nc.vector.tensor_copy(cell["ob"][:, qq, osl], po[:])

            def piece_d():
                nc.sync.dma_start(o_part[qb], cell["ob"][:])

            pieces = []
            for qq in range(4):
                pieces.append(lambda qq=qq: piece_a(qq))
                pieces.append(lambda qq=qq: piece_b(qq))
            for qq in range(4):
                for halfo in range(2):
                    pieces.append(
                        lambda qq=qq, h=halfo: piece_c(qq, h))
            pieces.append(piece_d)
            return pieces

        def sweep(qb, interleave):
            for kb in range(NKB):
                if pending_pv:
                    pending_pv.pop(0)()
                for fn in interleave.pop(kb, []):
                    fn()
                emit_scores(qb, kb, 0)
                emit_scores(qb, kb, 1)
                if (kb + 1) % GRP == 0:
                    pending_pv.extend(make_pv_closures(qb, kb // GRP))

        # ---------------- prologue: q projection + slab 0 ----------------
        xq_slabs = [load_xslab(xqT, s) for s in range(3)]
        for s in range(4):
            if s == 3:
                xq_slabs.append(load_xslab(xqT, 3))
            for i in range(2):
                proj_strip(xq_slabs[s], wq_sb, qT,
                           slice(s * 512, (s + 1) * 512), i)
        xk0 = load_xslab(xkT, 0)
        xv0 = load_xslab(xvT, 0)
        for i in range(2):
            proj_strip(xk0, wk_sb, kT, slice(0, 512), i)
        for r in range(4):
            v_group(xv0, 0, r)

        # qb0: K/V slabs 1..7 spread just-in-time across the k-loop
        inter0 = {}
        slabs = {}

        def k_strip(s, i):
            proj_strip(slabs[("k", s)], wk_sb, kT,
                       slice(s * 512, (s + 1) * 512), i)

        for s in range(1, 8):
            inter0.setdefault(4 * s - 4, []).append(
                lambda s=s: slabs.__setitem__(("k", s), load_xslab(xkT, s)))
            inter0.setdefault(4 * s - 3, []).append(
                lambda s=s: slabs.__setitem__(("v", s), load_xslab(xvT, s)))
            inter0.setdefault(4 * s - 2, []).append(lambda s=s: k_strip(s, 0))
            inter0.setdefault(4 * s - 1, []).append(lambda s=s: k_strip(s, 1))
            for r in range(4):
                inter0.setdefault(4 * s + r, []).append(
                    lambda s=s, r=r: v_group(slabs[("v", s)], s, r))
        sweep(0, inter0)

        for qb in range(1, NQB):
            pieces = finish_pieces(qb - 1)
            inter = {1 + k: [p] for k, p in enumerate(pieces)}
            sweep(qb, inter)

        while pending_pv:
            pending_pv.pop(0)()
        for p in finish_pieces(NQB - 1):
            p()

        _pools.close()

    nc.compile()
    return nc


def _get_nc():
    global _CACHED_NC
    if _CACHED_NC is None:
        _CACHED_NC = _build()
    return _CACHED_NC


def _slabify(xT_bf16):
    """[HID, L] -> [L//512, 128, NCH, 512] (slab-major, contiguous DMA)."""
    L = xT_bf16.shape[1]
    return np.ascontiguousarray(
        xT_bf16.reshape(NCH, 128, L // 512, 512).transpose(2, 1, 0, 3))


def _wslab(w_bf16):
    """[HID, n] -> [128, NCH, n]."""
    n = w_bf16.shape[1]
    return np.ascontiguousarray(
        w_bf16.reshape(NCH, 128, n).transpose(1, 0, 2))


def make_in_maps(query, key, value, w_q, w_k, w_v, w_o):
    import ml_dtypes
    bf = ml_dtypes.bfloat16
    qs = SCALE  # fold the softmax scale into W_q
    xq = [_slabify(query[b].T.astype(bf)) for b in range(B)]
    xk = [_slabify(key[b].T.astype(bf)) for b in range(B)]
    xv = [_slabify(value[b].T.astype(bf)) for b in range(B)]
    eye = np.eye(128, dtype=np.float32)
    ins = []
    for core in range(NCORE):
        b, t = core // 4, core % 4
        csl = slice(t * TC, (t + 1) * TC)
        ins.append({
            "xqT": xq[b],
            "xkT": xk[b],
            "xvT": xv[b],
            "wq": _wslab((w_q[:, csl] * qs).astype(bf)),
            "wk": _wslab(w_k[:, csl].astype(bf)),
            "wv": _wslab(w_v[:, csl].astype(bf)),
            "wo": np.ascontiguousarray(
                w_o[csl, :].astype(bf).reshape(2, 128, HID)
                .transpose(1, 0, 2)),
            "eye": eye,
        })
    return ins


def assemble(results):
    out = np.empty((B, Q, HID), np.float32)
    for b in range(B):
        acc = results[b * 4]["o_part"].astype(np.float32)
        for t in range(1, 4):
            acc += results[b * 4 + t]["o_part"].astype(np.float32)
        # o_part[qb, p, qq, :] -> rows qb*512 + qq*128 + p
        out[b] = acc.transpose(0, 2, 1, 3).reshape(Q, HID)
    return out


_EXEC = None


def _get_exec():
    """Build the 8-core shard_map executable once; reuse across calls."""
    global _EXEC
    if _EXEC is not None:
        return _EXEC
    import jax
    from jax.sharding import Mesh, PartitionSpec
    from jax.experimental.shard_map import shard_map
    from concourse.bass2jax import (_bass_exec_p, install_neuronx_cc_hook,
                                    partition_id_tensor)

    install_neuronx_cc_hook()
    nc = _get_nc()
    in_names, out_names, out_avals, zero_outs = [], [], [], []
    for alloc in nc.m.functions[0].allocations:
        if not isinstance(alloc, mybir.MemoryLocationSet):
            continue
        name = alloc.memorylocations[0].name
        if alloc.kind == "ExternalInput":
            if name != "partition_id":
                in_names.append(name)
        elif alloc.kind == "ExternalOutput":
            out_names.append(name)
            shape = tuple(alloc.tensor_shape)
            dtype = mybir.dt.np(alloc.dtype)
            out_avals.append(jax.core.ShapedArray(shape, dtype))
            zero_outs.append(np.zeros(shape, dtype))
    partition_name = (nc.partition_id_tensor.name
                      if nc.partition_id_tensor else None)
    all_in = list(in_names) + list(out_names)
    if partition_name:
        all_in.append(partition_name)

    def _body(*args):
        operands = list(args)
        if partition_name is not None:
            operands.append(partition_id_tensor())
        return tuple(_bass_exec_p.bind(
            *operands, out_avals=tuple(out_avals), in_names=tuple(all_in),
            out_names=tuple(out_names), lowering_input_output_aliases=(),
            sim_require_finite=True, sim_require_nnan=True, nc=nc))

    devices = jax.devices()[:NCORE]
    mesh = Mesh(np.asarray(devices), ("core",))
    n_all = len(in_names) + len(out_names)
    fn = jax.jit(shard_map(_body, mesh=mesh,
                           in_specs=(PartitionSpec("core"),) * n_all,
                           out_specs=(PartitionSpec("core"),) * len(out_names),
                           check_rep=False), keep_unused=True)
    concat_zeros = [np.zeros((NCORE * z.shape[0], *z.shape[1:]), z.dtype)
                    for z in zero_outs]
    _EXEC = (fn, in_names, out_names, out_avals, concat_zeros)
    return _EXEC


def kernel(query, key, value, w_q, w_k, w_v, w_o):
    query = np.asarray(query, dtype=np.float32)
    key = np.asarray(key, dtype=np.float32)
    value = np.asarray(value, dtype=np.float32)
    ins = make_in_maps(query, key, value, np.asarray(w_q, np.float32),
                       np.asarray(w_k, np.float32),
                       np.asarray(w_v, np.float32),
                       np.asarray(w_o, np.float32))
    fn, in_names, out_names, out_avals, concat_zeros = _get_exec()
    concat_in = [np.concatenate([np.asarray(ins[c][nm]) for c in range(NCORE)])
                 for nm in in_names]
    out_arrs = fn(*concat_in, *concat_zeros)
    results = [
        {nm: np.asarray(out_arrs[i]).reshape(NCORE, *out_avals[i].shape)[c]
         for i, nm in enumerate(out_names)}
        for c in range(NCORE)]
    return assemble(results)


if __name__ == "__main__":
    np.random.seed(0)
    q = np.random.randn(B, Q, HID).astype(np.float32)
    k = np.random.randn(B, KL, HID).astype(np.float32)
    v = np.random.randn(B, KL, HID).astype(np.float32)
    s = 1.0 / np.sqrt(HID)
    wq_ = (np.random.randn(HID, HID) * s).astype(np.float32)
    wk_ = (np.random.randn(HID, HID) * s).astype(np.float32)
    wv_ = (np.random.randn(HID, HID) * s).astype(np.float32)
    wo_ = (np.random.randn(HID, HID) * s).astype(np.float32)
    t0 = time.time()
    out = kernel(q, k, v, wq_, wk_, wv_, wo_)
    print("kernel done", time.time() - t0, out.shape)
